# revision 43
# baseline (speedup 1.0000x reference)
"""Trainium2 Bass kernel v3 for nn_MultiHeadSSAN.

Changes vs v2:
- Band scan moved from DVE (tensor_tensor_scan + strided assemble, ~650us
  DVE-bound phase) to the PE: host ships difference sequences
  h = psq + uq - shift(uq) packed [128=(half,l), m, 2, S]; an inclusive-scan
  matmul against a static block-diagonal upper-triangular ones matrix
  produces banded Q/K directly (2 chunks of 64 l's packed per matmul).
- V projection moved into the attention loop (no vpdram bounce: -33MB DMA).
- V bias folded into the out-proj bias host-side (attn rows sum to 1).
- Out-proj bias via ACT Identity-bias / DVE tensor_scalar_add during evac
  (drops the bias matmuls).
- Denominator via static ones-column matmul -> [1,2S] row in PSUM ->
  broadcast matmul -> reciprocal_approx_fast (kills the per-head N=1
  matmul + transpose + rrow-DMA chain).
- Pass-1 max chain batched per n: scores for 2 heads per PSUM bank, 3D
  tensor_reduce, one cast + one PE transpose + one lse DMA per n.
"""
import math
from contextlib import ExitStack

import numpy as np

import concourse.bass as bass
import concourse.bacc as bacc
import concourse.mybir as mybir
import concourse.tile as tile
from concourse.bass_utils import run_bass_kernel_spmd

F32 = mybir.dt.float32
F16 = mybir.dt.float16
ALU = mybir.AluOpType
ACTF = mybir.ActivationFunctionType
AX = mybir.AxisListType


class Cfg:
    def __init__(self, S=256, L=512, E=512, H=4, NC=8, OFF=4,
                 NPAIR=8, skip_band=False, skip_attn=False, nmax=None,
                 dbg_qk=False, tune=None):
        self.dbg_qk = dbg_qk
        self.S, self.L, self.E, self.H, self.NC = S, L, E, H, NC
        self.CH = L // NC
        self.OFF = OFF
        assert OFF * 2 >= NC
        self.HD = E // H
        assert self.HD == 128
        self.EB = E // 128
        self.NST = (S + 127) // 128
        self.STW = min(128, S)
        self.NPAIR = NPAIR
        assert self.CH % NPAIR == 0
        self.skip_band = skip_band
        self.skip_attn = skip_attn
        self.nmax = nmax if nmax is not None else self.CH
        self.tune = dict(ps_a=2, ps_b=2, ps_t=1, ps_v=1, qk_bufs=2,
                         xt_bufs=1, qp_bufs=5, vp_bufs=10, pt_bufs=12,
                         osc_bufs=8, bh_bufs=4, bo_bufs=3, ps_s=8,
                         rs_bufs=3, lr_bufs=3, sm_bufs=10, oo_bufs=3,
                         mgrp=8)
        if tune:
            self.tune.update(tune)

    def key(self):
        return (self.S, self.L, self.E, self.H, self.NC, self.OFF,
                self.NPAIR, self.skip_band, self.skip_attn, self.nmax,
                self.dbg_qk, tuple(sorted(self.tune.items())))


def build_nc(cfg: Cfg) -> bass.Bass:
    S, L, E, H, NC = cfg.S, cfg.L, cfg.E, cfg.H, cfg.NC
    CH, EB, HD = cfg.CH, cfg.EB, cfg.HD
    NST, STW = cfg.NST, cfg.STW
    NP = cfg.NPAIR
    T = cfg.tune
    NM = E // 4                      # number of e-quads (band matmuls/side)
    MG = T["mgrp"]                   # band m's per DMA load
    assert MG == 8, "band store perm in prep_inputs assumes mgrp=8"

    nc = bacc.Bacc(None)
    # ---- parameters
    xattn = nc.declare_dram_parameter("xattn", [E, CH, S], F16, isOutput=False)
    hq_in = nc.declare_dram_parameter("hq", [128, NM, 2, S], F16, isOutput=False)
    hk_in = nc.declare_dram_parameter("hk", [128, NM, 2, S], F16, isOutput=False)
    utblk_in = nc.declare_dram_parameter("utblk", [128, 128], F16, isOutput=False)
    wq = nc.declare_dram_parameter("wq", [E, E], F16, isOutput=False)
    wk = nc.declare_dram_parameter("wk", [E, E], F16, isOutput=False)
    wv = nc.declare_dram_parameter("wv", [E, E], F16, isOutput=False)
    wo = nc.declare_dram_parameter("wo", [E, E], F16, isOutput=False)
    bqin = nc.declare_dram_parameter("bqin", [E, S], F32, isOutput=False)
    bkin = nc.declare_dram_parameter("bkin", [E, S], F32, isOutput=False)
    biasc = nc.declare_dram_parameter("biasc", [E, 4], F32, isOutput=False)
    ident_in = nc.declare_dram_parameter("ident_in", [128, 128], F16, isOutput=False)
    out = nc.declare_dram_parameter("out", [E, CH, S], F16, isOutput=True)

    # ---- internal DRAM: banded q/k, l-major so band stores are one
    # contiguous 8KB run per partition; e rows are perm-space
    if cfg.dbg_qk:
        qkdram = nc.declare_dram_parameter("qkdump", [CH, 2, E, S], F16,
                                           isOutput=True)
    else:
        qkdram = nc.dram_tensor("qkdram", [CH, 2, E, S], F16)

    with tile.TileContext(nc) as tc:
        with tc.tile_pool(name="const", bufs=1) as cpool:
            ident = cpool.tile([128, 128], F16, name="ident")
            nc.sync.dma_start(ident[:], ident_in[:, :])
            scanw = cpool.tile([128, 128], F16, name="scanw")
            nc.sync.dma_start(scanw[:], utblk_in[:, :])
            ones16 = cpool.tile([1, max(S, 512)], F16, name="ones16")
            nc.vector.memset(ones16[:], 1.0)
            onesc16 = cpool.tile([128, 8], F16, name="onesc16")
            nc.vector.memset(onesc16[:], 1.0)
            biasc_sb = cpool.tile([128, 4 * EB], F32, name="biasc_sb")
            for eb in range(EB):
                nc.sync.dma_start(biasc_sb[:, 4 * eb:4 * (eb + 1)],
                                  biasc[eb * 128:(eb + 1) * 128, :])

            def load_w(dram, nm):
                tiles = []
                for eb in range(EB):
                    t = cpool.tile([128, E], F16, name=f"{nm}_{eb}")
                    nc.sync.dma_start(t[:], dram[eb * 128:(eb + 1) * 128, :])
                    tiles.append(t)
                return tiles

            wq_sb = load_w(wq, "wq")
            wk_sb = load_w(wk, "wk")
            wv_sb = load_w(wv, "wv")
            wo_sb = load_w(wo, "wo")

            # ---- B projection (host-computed boundary sums -> proj domain)
            def bproj(bdram, w_sb, bias_j, nm, bpool, psB):
                b16 = []
                for eb in range(EB):
                    t32 = bpool.tile([128, S], F32, name="B32", tag="B32")
                    nc.sync.dma_start(t32[:], bdram[eb * 128:(eb + 1) * 128, :])
                    t16 = bpool.tile([128, S], F16, name="B16", tag=f"B16{nm}{eb}")
                    nc.vector.tensor_copy(t16[:], t32[:])
                    b16.append(t16)
                out2 = []
                for fm in range(EB):
                    fr = slice(fm * 128, (fm + 1) * 128)
                    acc = psB.tile([128, S], F32, name=f"psB{nm}{fm}", tag="psB")
                    for eb in range(EB):
                        nc.tensor.matmul(acc[:], w_sb[eb][:, fr], b16[eb][:],
                                         start=(eb == 0), stop=(eb == EB - 1))
                    o32 = bpool.tile([128, S], F32, name="Bp32", tag="Bp32")
                    nc.vector.tensor_scalar_add(
                        o32[:], acc[:],
                        biasc_sb[:, 4 * fm + bias_j:4 * fm + bias_j + 1])
                    o2 = cpool.tile([128, 2 * S], F16, name=f"Bp2{nm}{fm}")
                    nc.vector.tensor_copy(o2[:, 0:S], o32[:])
                    nc.vector.tensor_copy(o2[:, S:2 * S], o32[:])
                    out2.append(o2)
                return out2

            with (tc.tile_pool(name="bpp", bufs=2) as bpool,
                  tc.tile_pool(name="psB", bufs=2, space="PSUM") as psB):
                Bqp2 = bproj(bqin, wq_sb, 0, "q", bpool, psB)
                Bkp2 = bproj(bkin, wk_sb, 1, "k", bpool, psB)

            # ================= band: scan via PE matmul =================
            # qband[l] = sum_{l'<=l} h[l'] with h = psq + uq - shift(uq).
            # Two 64-l chunks of e-quads packed per matmul via a block-diag
            # upper-triangular ones weight (static -> one LDWEIGHTS).
            if not cfg.skip_band:
                ev_flip = [0]
                with (
                    tc.tile_pool(name="bh", bufs=T["bh_bufs"]) as bhpool,
                    tc.tile_pool(name="bo", bufs=T["bo_bufs"]) as bopool,
                    tc.tile_pool(name="psS", bufs=T["ps_s"], space="PSUM") as psS,
                ):
                    for qki, hdram in ((0, hq_in), (1, hk_in)):
                        for mg in range(NM // MG):
                            ht = bhpool.tile([128, MG * 2 * S], F16,
                                             name="ht", tag="ht")
                            nc.sync.dma_start(
                                ht[:], hdram[:, mg * MG:(mg + 1) * MG, :, :])
                            ht3 = ht[:].rearrange("p (m c) -> p m c", m=MG)
                            stg = bopool.tile([128, MG * 2 * S], F16,
                                              name="stg", tag="stg")
                            for mi in range(MG):
                                acc = psS.tile([128, 2 * S], F32, name="psS",
                                               tag="psS")
                                nc.tensor.matmul(acc[:], scanw[:],
                                                 ht3[:, mi, :],
                                                 start=True, stop=True)
                                sl = stg[:, mi * 2 * S:(mi + 1) * 2 * S]
                                ev_flip[0] = (ev_flip[0] + 1) % 3
                                if ev_flip[0] == 0:
                                    nc.scalar.activation(sl, acc[:], ACTF.Copy)
                                else:
                                    nc.vector.tensor_copy(sl, acc[:])
                            # one batched store per 64-partition half. qkdram
                            # e-rows are PERMUTED (row r = e 32*mg+4*j+2*hh+i
                            # at r = 32*mg+16*hh+2*j+i); host permutes wq/wk
                            # and Bq/Bk rows to match, so each half is a
                            # contiguous 16-row block = one 8KB run/partition.
                            for hh in range(2):
                                r0 = 4 * MG * mg + 2 * MG * hh
                                dst = qkdram[:, qki, r0:r0 + 2 * MG, :]
                                nc.scalar.dma_start(
                                    dst,
                                    stg[64 * hh:64 * hh + 64, :]
                                    .rearrange("p (e s) -> p e s", e=2 * MG))

            # ================= attention =================
            if not cfg.skip_attn:
                with ExitStack() as apools:
                    ent = apools.enter_context
                    qkpool = ent(tc.tile_pool(name="qk", bufs=T["qk_bufs"]))
                    xtpool = ent(tc.tile_pool(name="xt", bufs=T["xt_bufs"]))
                    qppool = ent(tc.tile_pool(name="qp", bufs=T["qp_bufs"]))
                    kppool = ent(tc.tile_pool(name="kp", bufs=T["qp_bufs"]))
                    vppool = ent(tc.tile_pool(name="vp", bufs=T["vp_bufs"]))
                    ptpool = ent(tc.tile_pool(name="pt", bufs=T["pt_bufs"]))
                    oscpool = ent(tc.tile_pool(name="osc", bufs=T["osc_bufs"]))
                    oopool = ent(tc.tile_pool(name="oo", bufs=T["oo_bufs"]))
                    smpool = ent(tc.tile_pool(name="sm", bufs=T["sm_bufs"]))
                    rspool = ent(tc.tile_pool(name="rs", bufs=T["rs_bufs"]))
                    lrpool = ent(tc.tile_pool(name="lr", bufs=T["lr_bufs"]))
                    ps_a = ent(tc.tile_pool(name="ps_a", bufs=T["ps_a"],
                                            space="PSUM"))
                    ps_b = ent(tc.tile_pool(name="ps_b", bufs=T["ps_b"],
                                            space="PSUM"))
                    ps_t = ent(tc.tile_pool(name="ps_t", bufs=T["ps_t"],
                                            space="PSUM"))
                    ps_v = ent(tc.tile_pool(name="ps_v", bufs=T["ps_v"],
                                            space="PSUM"))
                    copy_flip = [0]
                    osc_box = [None]

                    def evac(dst, src):
                        # alternate plain PSUM->SBUF copies between DVE and ACT
                        copy_flip[0] ^= 1
                        if copy_flip[0]:
                            nc.vector.tensor_copy(dst, src)
                        else:
                            nc.scalar.activation(dst, src, ACTF.Copy)

                    NQ = (cfg.nmax + NP - 1) // NP
                    for g in range(NQ):
                        n0 = g * NP
                        qk_t = []
                        for eb in range(EB):
                            er = slice(eb * 128, (eb + 1) * 128)
                            t = qkpool.tile([128, NP * 2 * S], F16,
                                            name=f"qk{eb}", tag=f"qk{eb}")
                            nc.sync.dma_start(
                                t[:],
                                qkdram[n0:n0 + NP, :, er, :]
                                .rearrange("n q e s -> e (n q) s"))
                            qk_t.append(t)
                        xt_b = []
                        for eb in range(EB):
                            er = slice(eb * 128, (eb + 1) * 128)
                            t = xtpool.tile([128, NP * S], F16,
                                            name=f"xtb{eb}", tag=f"xtb{eb}")
                            nc.sync.dma_start(t[:], xattn[er, n0:n0 + NP, :])
                            xt_b.append(t)

                        def qk_view(eb, half, qki):
                            # (p, n2, s) view of pair `half`'s q or k columns
                            v = qk_t[eb][:].rearrange(
                                "p (n two s) -> p n two s", two=2, s=S)
                            return v[:, 2 * half:2 * half + 2, qki, :]

                        # q/k projections per pair-half at N=512
                        qp_t, kp_t = [], []
                        for fm in range(EB):
                            fr = slice(fm * 128, (fm + 1) * 128)
                            for (w_sb, B2, dst, nm) in (
                                    (wq_sb, Bqp2, qp_t, "qp"),
                                    (wk_sb, Bkp2, kp_t, "kp")):
                                tdst = (qppool if nm == "qp" else kppool).tile(
                                    [128, NP * S], F16, name=f"{nm}{fm}", tag=nm)
                                for half in range(NP // 2):
                                    acc = ps_a.tile([128, 2 * S], F32,
                                                    name=f"ps{nm}", tag="ps_mm")
                                    for eb in range(EB):
                                        nc.tensor.matmul(
                                            acc[:], w_sb[eb][:, fr],
                                            qk_view(eb, half, 0 if nm == "qp" else 1),
                                            start=(eb == 0), stop=(eb == EB - 1))
                                    # B-term (boundary sums + q/k bias) added
                                    # during the DVE evac instead of an extra
                                    # ident matmul on the PE
                                    nc.any.tensor_tensor(
                                        tdst[:, half * 2 * S:(half + 1) * 2 * S],
                                        acc[:], B2[fm][:], op=ALU.add)
                                dst.append(tdst)

                        # V projection for this group (from xattn, no bias:
                        # v-bias is folded into the out-proj bias host-side)
                        vp_t = [[None] * NST for _ in range(NP)]
                        for jj in range(NP):
                            for st in range(NST):
                                scols = slice(jj * S + st * 128,
                                              jj * S + st * 128 + STW)
                                acc = ps_v.tile([STW, E], F32, name="psv",
                                                tag="psv")
                                for eb in range(EB):
                                    nc.tensor.matmul(acc[:], xt_b[eb][:, scols],
                                                     wv_sb[eb][:],
                                                     start=(eb == 0),
                                                     stop=(eb == EB - 1))
                                o = vppool.tile([STW, E], F16, name="vp",
                                                tag="vp")
                                nc.any.tensor_copy(o[:], acc[:])
                                vp_t[jj][st] = o

                        # per-n work is split into pass1 (scores -> -max row)
                        # and stage2 (everything after); pass1(j+1) is
                        # emitted before stage2(j) so the PE has work queued
                        # while the lse-row chain of n j completes.
                        def pass1(j):
                            ncols = slice(j * S, (j + 1) * S)
                            # ---- pass 1: -max per (s-row, head) -> lse row
                            nmax8 = smpool.tile([STW, 2 * H], F32,
                                                name="nmax8", tag="nmax8")
                            for st in range(NST):
                                qcols = slice(j * S + st * 128,
                                              j * S + st * 128 + STW)
                                for hp in range(2):
                                    acc = ps_b.tile([STW, 2 * S], F32,
                                                    name="pssc", tag="ps_sc")
                                    for h2 in range(2):
                                        h = 2 * hp + h2
                                        nc.tensor.matmul(
                                            acc[:, h2 * S:(h2 + 1) * S],
                                            qp_t[h][:, qcols],
                                            kp_t[h][:, ncols],
                                            start=True, stop=True)
                                    # nmax8 cols ordered (h, st) so the lse
                                    # row DMA below is a straight copy
                                    nv = nmax8[:].rearrange(
                                        "p (h st) -> p h st", st=NST)
                                    nc.vector.tensor_reduce(
                                        nv[:, 2 * hp:2 * hp + 2, st],
                                        acc[:].rearrange("p (g t) -> p g t",
                                                         g=2),
                                        axis=AX.X, op=ALU.max, negate=True)
                            m16 = smpool.tile([STW, 2 * H], F16, name="m16",
                                              tag="m16")
                            nc.any.tensor_copy(m16[:], nmax8[:])
                            tp = ps_t.tile([2 * H, STW], F16, name="pstp",
                                           tag="ps_small")
                            nc.tensor.transpose(tp[:], m16[:], ident[:])
                            l4 = smpool.tile([2 * H, STW], F16, name="l4",
                                             tag="l4")
                            nc.any.tensor_copy(l4[:], tp[:])
                            lrow = lrpool.tile([1, H * S], F16, name="lrow",
                                               tag="lrow")
                            # src rows iterate (h, st); dest (h, st, s)
                            nc.sync.dma_start(
                                lrow[0:1, :].rearrange(
                                    "o (r s) -> o r s", r=2 * H),
                                l4[:])
                            return lrow

                        def stage2(j, lrow):
                            ncols = slice(j * S, (j + 1) * S)
                            # ---- pass 2: scores^T - max -> exp (unnormalized)
                            PT = [[None] * NST for _ in range(2)]
                            for hp in range(2):
                                for tt in range(NST):
                                    tcols = slice(j * S + tt * 128,
                                                  j * S + tt * 128 + STW)
                                    acc = ps_b.tile([STW, 2 * S], F32,
                                                    name="psT", tag="ps_sc")
                                    for h2 in range(2):
                                        h = 2 * hp + h2
                                        csl = slice(h2 * S, (h2 + 1) * S)
                                        nc.tensor.matmul(
                                            acc[:, csl], kp_t[h][:, tcols],
                                            qp_t[h][:, ncols],
                                            start=True, stop=False)
                                        nc.tensor.matmul(
                                            acc[:, csl], ones16[:1, :STW],
                                            lrow[0:1, h * S:(h + 1) * S],
                                            start=False, stop=True)
                                    p = ptpool.tile([STW, 2 * S], F16,
                                                    name="PT", tag="PT")
                                    nc.scalar.activation(p[:], acc[:], ACTF.Exp)
                                    PT[hp][tt] = p

                            # ---- den rows via static ones-col matmul; 1/den
                            # broadcast via rep matmul + fast reciprocal
                            rs32 = [None, None]
                            for hp in range(2):
                                # den row shares the rep bank: row 0 is
                                # written, copied out, then overwritten by
                                # the broadcast matmul (WAR-tracked).
                                rep = ps_t.tile([STW, 2 * S], F32, name="psrep",
                                                tag="ps_rep")
                                for tt in range(NST):
                                    nc.tensor.matmul(
                                        rep[0:1, :], onesc16[:STW, 0:1],
                                        PT[hp][tt][:],
                                        start=(tt == 0), stop=(tt == NST - 1))
                                denrow = smpool.tile([1, 2 * S], F16,
                                                     name="denrow", tag="denrow")
                                nc.any.tensor_copy(denrow[:], rep[0:1, :])
                                nc.tensor.matmul(rep[:], ones16[:1, :STW],
                                                 denrow[0:1, :],
                                                 start=True, stop=True)
                                r = rspool.tile([STW, 2 * S], F32, name="rs32",
                                                tag="rs32")
                                nc.vector.reciprocal_approx_fast(r[:], rep[:])
                                rs32[hp] = r

                            # ---- attn @ V, normalize via 1/den broadcast
                            half = j // 2
                            jj = j % 2
                            if jj == 0:
                                osc_box[0] = [oscpool.tile([HD, 2 * S], F16,
                                                           name=f"osc{h}",
                                                           tag="osc")
                                              for h in range(H)]
                            osc_cur = osc_box[0]
                            for h in range(H):
                                hp, h2 = h // 2, h % 2
                                acc = ps_t.tile([HD, S], F32, name="pso",
                                                tag="ps_oo")
                                hr = slice(h * HD, (h + 1) * HD)
                                for tt in range(NST):
                                    nc.tensor.matmul(
                                        acc[:], vp_t[j][tt][:, hr],
                                        PT[hp][tt][:, h2 * S:(h2 + 1) * S],
                                        start=(tt == 0), stop=(tt == NST - 1))
                                nc.any.tensor_tensor(
                                    osc_cur[h][:, jj * S:(jj + 1) * S],
                                    acc[:],
                                    rs32[hp][:, h2 * S:(h2 + 1) * S],
                                    op=ALU.mult)

                            # ---- out projection per completed pair
                            if jj == 1 or n0 + j == cfg.nmax - 1:
                                width = (jj + 1) * S
                                for gm in range(EB):
                                    gr = slice(gm * 128, (gm + 1) * 128)
                                    acc = ps_a.tile([128, 2 * S], F32,
                                                    name="psout", tag="ps_mm")
                                    for fm in range(EB):
                                        nc.tensor.matmul(
                                            acc[:, :width], wo_sb[fm][:, gr],
                                            osc_cur[fm][:, :width],
                                            start=(fm == 0),
                                            stop=(fm == EB - 1))
                                    o = oopool.tile([128, 2 * S], F16, name="oo",
                                                    tag="oo")
                                    bcol = biasc_sb[:, 4 * gm + 3:4 * gm + 4]
                                    nc.any.tensor_scalar_add(
                                        o[:, :width], acc[:, :width], bcol)
                                    nc.scalar.dma_start(
                                        out[gr, n0 + 2 * half:
                                            n0 + 2 * half + (jj + 1), :],
                                        o[:, :width])

                        js = [j for j in range(NP) if n0 + j < cfg.nmax]
                        lrows = {}
                        for idx, j in enumerate(js):
                            lrows[j] = pass1(j)
                            if idx > 0:
                                stage2(js[idx - 1], lrows.pop(js[idx - 1]))
                        if js:
                            stage2(js[-1], lrows.pop(js[-1]))

    nc.finalize()
    return nc


# ============================================================
# host side
# ============================================================

def prep_inputs(cfg: Cfg, x, a, b, c, d, in_proj_w, in_proj_b, out_w, out_b):
    S, L, E, NC, CH, OFF = cfg.S, cfg.L, cfg.E, cfg.NC, cfg.CH, cfg.OFF
    f16, f32 = np.float16, np.float32
    x = np.asarray(x, f32)
    xg = np.ascontiguousarray(x.transpose(2, 0, 1))       # (E, S, L) fp32
    scl = 1.0 / math.sqrt(cfg.HD)
    # qband e-space permutation: qkdram row r = 32*mg+16*hh+2*j+i holds
    # original e = 32*mg+4*j+2*hh+i (band-store batching); wq/wk and Bq/Bk
    # rows (the q/k contraction inputs) follow the same order.
    MGK = 8
    perm = np.array([32 * mg + 4 * j + 2 * hh + i
                     for mg in range(E // (4 * MGK))
                     for hh in range(2)
                     for j in range(MGK)
                     for i in range(2)], np.int64)
    wq_h = np.ascontiguousarray(in_proj_w[:E].T * scl)[perm].astype(f16)
    wk_h = np.ascontiguousarray(in_proj_w[E:2 * E].T)[perm].astype(f16)
    wv_h = np.ascontiguousarray(in_proj_w[2 * E:].T).astype(f16)
    wo_h = np.ascontiguousarray(out_w.T).astype(f16)
    bq = in_proj_b[:E] * scl
    bk = in_proj_b[E:2 * E]
    bv = in_proj_b[2 * E:]
    # attention rows sum to 1 -> v-bias passes through attn unchanged;
    # fold it into the out-proj bias: bo2 = out_b + out_w @ bv
    bo2 = np.asarray(out_b, f32) + np.asarray(out_w, f32) @ np.asarray(bv, f32)
    biasc = np.ascontiguousarray(
        np.stack([bq, bk, np.zeros_like(bo2), bo2]).T).astype(f32)
    ident = np.eye(128, dtype=f16)

    # block-diag upper-triangular ones (inclusive scan weight), 2x64
    ut = np.triu(np.ones((CH, CH), f32))
    utblk = np.zeros((128, 128), f32)
    utblk[:CH, :CH] = ut
    utblk[CH:, CH:] = ut
    utblk = utblk.astype(f16)

    # full elementwise products in (E, S, L) layout
    aT, bT, cT, dT = (np.asarray(w, f32).T[:, None, :] for w in (a, b, c, d))
    XA = xg * aT
    XB = xg * bT
    XC = xg * cT
    XD = xg * dT

    # chunk totals (E, S) per chunk: T_w[j] = sum_l XW[:, :, chunk j]
    def tot(XW):
        return XW.reshape(E, S, NC, CH).sum(axis=3)       # (E, S, NC)
    Ta, Tb, Tc, Td = tot(XA), tot(XB), tot(XC), tot(XD)

    def pack_h(psq, uq):
        # h = psq + uq - shift(uq): inclusive scan of h gives uq + cumsum(psq)
        h = psq.copy()
        h += uq
        h[:, :, 1:] -= uq[:, :, :-1]
        # (E, S, CH) -> [(half,l)=128, m=E/4, i=2, s=S]
        A = h.transpose(2, 0, 1)                          # (CH, E, S)
        A = A.reshape(CH, E // 4, 2, 2, S)                # l, m, half, i, s
        return np.ascontiguousarray(
            A.transpose(2, 0, 1, 3, 4).reshape(2 * CH, E // 4, 2, S)
        ).astype(f16)

    in_maps = []
    for k in range(NC):
        chk = slice(CH * k, CH * (k + 1))
        xattnc = np.ascontiguousarray(
            xg[:, :, chk].transpose(0, 2, 1)).astype(f16)
        psq = XA[:, :, chk] - XC[:, :, chk]
        psk = XB[:, :, chk] - XD[:, :, chk]
        uq = xg[:, :, chk] - XA[:, :, chk]
        uk = xg[:, :, chk] - XB[:, :, chk]
        if k >= OFF:
            pf = slice(CH * (k - OFF), CH * (k - OFF + 1))
            psq = psq - XA[:, :, pf]
            psk = psk - XB[:, :, pf]
        else:
            st = CH * (k + OFF) - 1
            psq = psq.copy()
            psk = psk.copy()
            psq[:, :, 1:] += XC[:, :, st + 1:st + CH]
            psk[:, :, 1:] += XD[:, :, st + 1:st + CH]
        hq = pack_h(psq, uq)
        hk = pack_h(psk, uk)
        Bq = (Ta[:, :, max(0, k - OFF):k].sum(axis=2)
              + Tc[:, :, k:min(k + OFF - 1, NC - 1) + 1].sum(axis=2))
        Bk = (Tb[:, :, max(0, k - OFF):k].sum(axis=2)
              + Td[:, :, k:min(k + OFF - 1, NC - 1) + 1].sum(axis=2))
        in_maps.append(dict(
            hq=hq, hk=hk, utblk=utblk,
            xattn=xattnc,
            wq=wq_h, wk=wk_h, wv=wv_h, wo=wo_h,
            bqin=np.ascontiguousarray(Bq[perm]).astype(f32),
            bkin=np.ascontiguousarray(Bk[perm]).astype(f32),
            biasc=biasc,
            ident_in=ident,
        ))
    return in_maps


_CACHE = {}


def run(cfg: Cfg, inputs, core_ids=None, **kw):
    key = cfg.key()
    if key not in _CACHE:
        _CACHE[key] = build_nc(cfg)
    nc = _CACHE[key]
    in_maps = prep_inputs(
        cfg, inputs["x"], inputs["a"], inputs["b"], inputs["c"], inputs["d"],
        inputs["in_proj_w"], inputs["in_proj_b"], inputs["out_w"], inputs["out_b"])
    res = run_bass_kernel_spmd(nc, in_maps, core_ids or list(range(cfg.NC)), **kw)
    S, L, E, CH = cfg.S, cfg.L, cfg.E, cfg.CH
    full = np.empty((S, L, E), np.float32)
    for k in range(cfg.NC):
        full[:, CH * k:CH * (k + 1), :] = \
            res.results[k]["out"].astype(np.float32).transpose(2, 1, 0)
    return full, res


def kernel(**inputs) -> np.ndarray:
    assert int(inputs["n1"]) == 256 and int(inputs["n2"]) == 256
    cfg = Cfg()
    out, _ = run(cfg, inputs)
    return out
